# revision 1
# baseline (speedup 1.0000x reference)
"""Bass/Trainium2 kernel for the BayesianVectorRenderer problem.

Renders a closed cubic-Bezier path into a [1024,1024,4] RGBA image via a
soft winding-number accumulation.

Strategy (8 NeuronCores, SPMD):
  - Host: sample the Bezier path (512 points -> 512 edges).  The image is
    split into 64-row blocks; each core gets 2 blocks, greedily packed to
    balance work.  An edge is active for a core only where its validity
    window (t in ~[-0.6,1.6]) overlaps the core's rows (~110 of 512).
  - Decomposition (winding = FS - sum_j w_j*sigmoid(c - xc_j)):
      FS     = sum of all edge weights (winding at the far left),
      window = per edge, sigmoid(c - xc) differs from its limits only on
               [xcmin-16, xcmax+16]; each core sorts its edges by window
               center so the j-th slot's union window across cores is a
               TIGHT STATIC column range [S_j, B_j) (j-th order statistics
               of similar distributions align),
      step   = w * H(c - B_j) with B_j the slot's static 64-px boundary;
               steps sharing a boundary are mask-summed per partition once
               (tensor_tensor_reduce) and applied as ONE tail-add per
               bucket -- O(16) ops instead of O(edges).
  - ScalarEngine evaluates the sigmoid windows (per-partition bias = -xc),
    VectorEngine does fused multiply-accumulate (scalar_tensor_tensor),
    bucket reduces and tail adds.  Raw Bass with explicit semaphores
    (this walrus build rejects Tile's embedded on_wait encoding).
"""

from contextlib import ExitStack

import numpy as np

import concourse.bass as bass
from concourse import mybir
from concourse.bass_utils import run_bass_kernel_spmd

H = 1024
W = 1024
SAMPLES_PER_SEG = 32
N_CORES = 8
ROWS = H // N_CORES
BS = 64              # row-block size for load balancing
BKT = 64             # column bucket size for shared step-adds

T_LO = np.float32(-0.6)
T_HI = np.float32(1.6)
MARGIN = 16          # sigmoid(16) is within 1.2e-7 of 1

N_ACC = 2   # accumulators (breaks the DVE in-place RAW chain)
N_SIG = 16  # sigmoid buffer depth (ACT runs ahead of DVE)


def _sample_bezier(cp: np.ndarray) -> np.ndarray:
    """Faithful fp32 port of reference.sample_bezier_path."""
    cp = cp.astype(np.float32)
    n = cp.shape[0]
    s = (n - 1) // 3
    idx = 3 * np.arange(s)
    p0 = cp[idx][:, None, :]
    p1 = cp[idx + 1][:, None, :]
    p2 = cp[idx + 2][:, None, :]
    p3 = cp[idx + 3][:, None, :]
    t = np.linspace(0.0, 1.0, SAMPLES_PER_SEG, dtype=np.float32)[None, :, None]
    mt = (np.float32(1.0) - t).astype(np.float32)
    pts = (
        (mt * mt * mt) * p0
        + np.float32(3.0) * (mt * mt) * t * p1
        + np.float32(3.0) * mt * (t * t) * p2
        + (t * t * t) * p3
    )
    return pts.reshape(s * SAMPLES_PER_SEG, 2).astype(np.float32)


def _build_nc(starts, ends, buckets):
    """Build the SPMD Bass graph.

    Slot j's window is the static column range [starts[j], ends[j]);
    ends[j] is also the slot's step boundary (64-px aligned).  Slots are
    ordered so same-boundary slots are contiguous: buckets is a tuple of
    (B, slot_lo, slot_hi) ranges, each getting one slice-reduce and one
    shared tail-add over [B, W).
    Packed input [ROWS, 5e+4]: [inv | b | dx | x0 | cf | y | color]
    """
    e = len(starts)
    nb = len(buckets)
    wmax = max(en - st for st, en in zip(starts, ends))
    tot = 5 * e + 1 + 3
    nc = bass.Bass("TRN2", target_bir_lowering=False, debug=False)
    f32 = mybir.dt.float32
    A = mybir.AluOpType
    SIG = mybir.ActivationFunctionType.Sigmoid

    d_inp = nc.declare_dram_parameter("inp", [ROWS, tot], f32, isOutput=False)
    d_out = nc.declare_dram_parameter("out", [ROWS, W * 4], f32, isOutput=True)

    with ExitStack() as ctx:
        t_inp = ctx.enter_context(nc.sbuf_tensor([ROWS, tot], f32))
        t_b0 = ctx.enter_context(nc.sbuf_tensor([ROWS, 1], f32))
        t_b20 = ctx.enter_context(nc.sbuf_tensor([ROWS, 1], f32))
        t_fs = ctx.enter_context(nc.sbuf_tensor([ROWS, 1], f32))
        t_bs = ctx.enter_context(nc.sbuf_tensor([ROWS, max(nb, 1)], f32))
        t_grid = ctx.enter_context(nc.sbuf_tensor([ROWS, W], f32))
        t_t = ctx.enter_context(nc.sbuf_tensor([ROWS, e], f32))
        t_v1 = ctx.enter_context(nc.sbuf_tensor([ROWS, e], f32))
        t_v2 = ctx.enter_context(nc.sbuf_tensor([ROWS, e], f32))
        t_w = ctx.enter_context(nc.sbuf_tensor([ROWS, e], f32))
        t_wn = ctx.enter_context(nc.sbuf_tensor([ROWS, e], f32))
        t_xc = ctx.enter_context(nc.sbuf_tensor([ROWS, e], f32))
        t_nxc = ctx.enter_context(nc.sbuf_tensor([ROWS, e], f32))
        t_scr = ctx.enter_context(nc.sbuf_tensor([ROWS, e], f32))
        t_sig = ctx.enter_context(nc.sbuf_tensor([ROWS, N_SIG * wmax], f32))
        t_acc = ctx.enter_context(nc.sbuf_tensor([ROWS, N_ACC * W], f32))
        t_alpha = ctx.enter_context(nc.sbuf_tensor([ROWS, W], f32))
        t_rgba = ctx.enter_context(nc.sbuf_tensor([ROWS, W * 4], f32))
        dma_in = ctx.enter_context(nc.semaphore("dma_in"))
        pool_sem = ctx.enter_context(nc.semaphore("pool_sem"))
        dve_p1 = ctx.enter_context(nc.semaphore("dve_p1"))
        act_p = ctx.enter_context(nc.semaphore("act_p"))
        dve_p2 = ctx.enter_context(nc.semaphore("dve_p2"))
        act_loop = ctx.enter_context(nc.semaphore("act_loop"))
        dve_loop = ctx.enter_context(nc.semaphore("dve_loop"))
        act_alpha = ctx.enter_context(nc.semaphore("act_alpha"))
        dve_fin = ctx.enter_context(nc.semaphore("dve_fin"))
        dve_scr = ctx.enter_context(nc.semaphore("dve_scr"))
        dma_out = ctx.enter_context(nc.semaphore("dma_out"))
        block = ctx.enter_context(nc.Block())

        inp = t_inp[:]
        s_inv = inp[:, 0:e]
        s_b = inp[:, e:2 * e]
        s_dx = inp[:, 2 * e:3 * e]
        s_x0 = inp[:, 3 * e:4 * e]
        s_cf = inp[:, 4 * e:5 * e]
        s_y = inp[:, 5 * e:5 * e + 1]
        s_col = inp[:, 5 * e + 1:5 * e + 4]
        sig = [t_sig[:][:, k * wmax:(k + 1) * wmax] for k in range(N_SIG)]
        acc = [t_acc[:][:, k * W:(k + 1) * W] for k in range(N_ACC)]
        rgba4 = t_rgba[:].rearrange("p (c k) -> p c k", k=4)

        # dve_loop counting: 1 (acc0 init) + nb (tails) + e (loop) + 3 (folds)
        n_pre = 1 + nb
        n_loop = n_pre + e

        @block.sync
        def _(sync):
            sync.dma_start(out=inp, in_=d_inp[:]).then_inc(dma_in, 16)
            sync.wait_ge(dve_fin, 1)
            sync.wait_ge(pool_sem, 3)
            sync.dma_start(out=d_out[:], in_=t_rgba[:]).then_inc(dma_out, 16)

        @block.gpsimd
        def _(gpsimd):
            gpsimd.iota(
                t_grid[:], pattern=[[1, W]], base=0, channel_multiplier=0,
                allow_small_or_imprecise_dtypes=True,
            ).then_inc(pool_sem, 1)
            for k in range(1, N_ACC):
                gpsimd.memset(acc[k], 0.0)
            gpsimd.memset(acc[0], 0.0).then_inc(pool_sem, 1)
            # constant color channels of the output (Pool is otherwise idle)
            gpsimd.wait_ge(dma_in, 16)
            gpsimd.wait_ge(pool_sem, 1)  # own iota completed (t_grid RAW)
            for ch in range(2):
                gpsimd.tensor_scalar(
                    rgba4[:, :, ch], t_grid[:], 0.0, s_col[:, ch:ch + 1],
                    op0=A.mult, op1=A.add,
                )
            gpsimd.tensor_scalar(
                rgba4[:, :, 2], t_grid[:], 0.0, s_col[:, 2:3],
                op0=A.mult, op1=A.add,
            ).then_inc(pool_sem, 1)

        @block.vector
        def _(vector):
            vector.memset(t_b0[:], 0.0)
            vector.memset(t_b20[:], 20.0)
            vector.wait_ge(dma_in, 16)
            vector.scalar_tensor_tensor(
                t_t[:], s_inv, s_y, s_b, op0=A.mult, op1=A.subtract,
            ).then_inc(dve_p1, 1)
            vector.wait_ge(act_p, 1)
            vector.tensor_tensor(t_w[:], t_v1[:], t_v2[:], A.mult).then_inc(
                dve_scr, 1)
            vector.tensor_tensor(t_xc[:], t_t[:], s_dx, A.mult).then_inc(
                dve_scr, 1)
            vector.wait_ge(dve_scr, 2)
            vector.tensor_tensor(t_w[:], t_w[:], s_cf, A.mult).then_inc(
                dve_scr, 1)
            vector.tensor_tensor(t_xc[:], t_xc[:], s_x0, A.add).then_inc(
                dve_scr, 1)
            vector.wait_ge(dve_scr, 4)
            vector.tensor_scalar(t_wn[:], t_w[:], -1.0, None, op0=A.mult
                                 ).then_inc(dve_scr, 1)
            vector.tensor_scalar(
                t_nxc[:], t_xc[:], -1.0, None, op0=A.mult).then_inc(dve_p2, 1)
            vector.wait_ge(dve_scr, 5)
            # FS = sum of weights; per-bucket sums of -w
            vector.tensor_scalar(
                t_v1[:], t_w[:], 1.0, 0.0, op0=A.mult, op1=A.add,
                accum_out=t_fs[:, 0:1],
            ).then_inc(dve_scr, 1)
            for b, (bb_col, slo, shi) in enumerate(buckets):
                # disjoint t_scr/t_bs regions: no WAW chain needed
                vector.tensor_scalar(
                    t_scr[:, slo:shi], t_wn[:, slo:shi], 1.0, 0.0,
                    op0=A.mult, op1=A.add, accum_out=t_bs[:, b:b + 1],
                ).then_inc(dve_scr, 1)
            if nb == 0:
                vector.tensor_scalar(
                    t_bs[:, 0:1], t_fs[:, 0:1], 0.0, None, op0=A.mult,
                ).then_inc(dve_scr, 1)
            vector.wait_ge(dve_scr, 6 + max(nb, 1))
            vector.wait_ge(pool_sem, 2)
            # acc0 = FS
            vector.tensor_scalar(
                acc[0], acc[0], 0.0, t_fs[:, 0:1], op0=A.mult, op1=A.add,
            ).then_inc(dve_loop, 1)
            # shared step tail-adds (commutative: run BEFORE the window loop,
            # filling the DVE idle gap while ACT produces the first sigmoids)
            for b, (bb_col, slo, shi) in enumerate(buckets):
                vector.wait_ge(dve_loop, max(1, 1 + b - N_ACC + 1))
                vector.tensor_scalar(
                    acc[b % N_ACC][:, bb_col:W], acc[b % N_ACC][:, bb_col:W],
                    t_bs[:, b:b + 1], None, op0=A.add,
                ).then_inc(dve_loop, 1)
            for j in range(e):
                st, en = starts[j], ends[j]
                vector.wait_ge(act_loop, j + 1)
                vector.wait_ge(dve_loop, n_pre if j < N_ACC else n_pre + j - N_ACC + 1)
                a = acc[j % N_ACC]
                # acc[:, st:en] += sig * (-w_j)
                vector.scalar_tensor_tensor(
                    a[:, st:en], sig[j % N_SIG][:, 0:en - st],
                    t_wn[:, j:j + 1], a[:, st:en], op0=A.mult, op1=A.add,
                ).then_inc(dve_loop, 1)
            vector.wait_ge(dve_loop, n_loop)
            vector.tensor_tensor(acc[0], acc[0], acc[1], A.add).then_inc(dve_loop, 1)
            vector.wait_ge(act_alpha, 1)
            vector.tensor_copy(rgba4[:, :, 3], t_alpha[:]).then_inc(dve_fin, 1)

        @block.scalar
        def _(scalar):
            scalar.wait_ge(dve_p1, 1)
            scalar.activation(t_v1[:], t_t[:], SIG, bias=t_b0[:, 0:1], scale=20.0)
            scalar.activation(
                t_v2[:], t_t[:], SIG, bias=t_b20[:, 0:1], scale=-20.0,
            ).then_inc(act_p, 1)
            scalar.wait_ge(dve_p2, 1)
            scalar.wait_ge(pool_sem, 1)
            for j in range(e):
                st, en = starts[j], ends[j]
                if j >= N_SIG:
                    scalar.wait_ge(dve_loop, n_pre + j - N_SIG + 1)
                # sig = sigmoid(c - xc_j) over the slot's static window
                scalar.activation(
                    sig[j % N_SIG][:, 0:en - st], t_grid[:, st:en], SIG,
                    bias=t_nxc[:, j:j + 1], scale=1.0,
                ).then_inc(act_loop, 1)
            scalar.wait_ge(dve_loop, n_loop + 1)
            scalar.activation(
                t_alpha[:], acc[0], SIG, bias=t_b0[:, 0:1], scale=4.0,
            ).then_inc(act_alpha, 1)

    return nc


def _xc_at(x0, y0, inv, dx, y):
    return x0 + (y - y0) * inv * dx


def _prepare(control_points: np.ndarray, color: np.ndarray):
    """Host prep: plan the decomposition, build the graph + input maps."""
    cp = np.asarray(control_points, dtype=np.float32)
    col = np.asarray(color, dtype=np.float32)

    pts = _sample_bezier(cp)
    nxt = np.roll(pts, -1, axis=0)
    x0 = pts[:, 0]
    y0 = pts[:, 1]
    dy = nxt[:, 1] - y0
    dx = nxt[:, 0] - x0
    coeff = (np.sign(dy) * (np.abs(dy) >= np.float32(1e-6))).astype(np.float32)
    inv = (np.float32(1.0) / (dy + np.float32(1e-8))).astype(np.float32)
    b_arr = (y0 * inv).astype(np.float32)

    ya = y0 + T_LO * dy
    yb = y0 + T_HI * dy
    ymin = np.minimum(ya, yb)
    ymax = np.maximum(ya, yb)

    n_blocks = H // BS
    per_core = ROWS // BS
    blk_sets = []
    for b in range(n_blocks):
        r0 = b * BS
        s = np.nonzero((coeff != 0) & (ymax >= r0) & (ymin <= r0 + BS - 1))[0]
        blk_sets.append(set(s.tolist()))

    def window(j, blocks):
        """Static column window [lo, hi) of edge j over the blocks' rows."""
        xmn = xmx = None
        for b in blocks:
            r0, r1 = b * BS, b * BS + BS - 1
            a = max(ymin[j], r0)
            bb = min(ymax[j], r1)
            if a > bb:
                continue
            v0 = _xc_at(x0[j], y0[j], inv[j], dx[j], a)
            v1 = _xc_at(x0[j], y0[j], inv[j], dx[j], bb)
            lo, hi = min(v0, v1), max(v0, v1)
            xmn = lo if xmn is None else min(xmn, lo)
            xmx = hi if xmx is None else max(xmx, hi)
        if xmn is None:
            return None
        if not (np.isfinite(xmn) and np.isfinite(xmx)):
            return (0, W)
        if xmx + MARGIN <= 0:
            return None  # fully left of canvas: contributes ~0
        lo = int(np.clip(np.floor(xmn) - MARGIN, 0, W - 1))
        hi = int(np.clip(np.ceil(xmx) + MARGIN, lo + 1, W))
        return (lo, hi)

    # greedy pack 2 blocks per core, minimizing estimated engine time
    order = sorted(range(n_blocks), key=lambda i: -len(blk_sets[i]))
    core_edge_sets = [set() for _ in range(N_CORES)]
    core_blocks = [[] for _ in range(N_CORES)]
    for i in order:
        best, best_cost = None, None
        for c in range(N_CORES):
            if len(core_blocks[c]) >= per_core:
                continue
            u = core_edge_sets[c] | blk_sets[i]
            cost = 0
            for j in u:
                g = window(j, core_blocks[c] + [i])
                if g is not None:
                    cost += 300 + (g[1] - g[0])
            if best_cost is None or cost < best_cost:
                best_cost, best = cost, c
        core_edge_sets[best] |= blk_sets[i]
        core_blocks[best].append(i)

    # per-core edge windows, sorted by window center
    core_lists = []
    for c in range(N_CORES):
        lst = []
        for j in sorted(core_edge_sets[c]):
            g = window(j, core_blocks[c])
            if g is not None:
                lst.append((j, g[0], g[1]))
        lst.sort(key=lambda t: t[1] + t[2])
        core_lists.append(lst)

    e = max(8, int(np.ceil(max(len(l) for l in core_lists) / 8.0)) * 8)
    starts = [W] * e
    ends = [0] * e
    for c in range(N_CORES):
        core_lists[c] = core_lists[c][:e]
        for s, (j, lo, hi) in enumerate(core_lists[c]):
            starts[s] = min(starts[s], lo)
            ends[s] = max(ends[s], hi)
    # slot boundary: 64-px aligned end (also the step start)
    for s in range(e):
        if ends[s] == 0:  # pure padding slot, no real edge in any core
            starts[s], ends[s] = W - 8, W
            continue
        ends[s] = int(min(W, int(np.ceil(ends[s] / BKT)) * BKT))
        starts[s] = min(starts[s], ends[s] - 8)

    # permute slots so same-boundary slots are contiguous (bucket slices)
    perm = sorted(range(e), key=lambda s: (ends[s], s))
    starts = [starts[s] for s in perm]
    ends = [ends[s] for s in perm]
    inv_lists = []
    for c in range(N_CORES):
        old = core_lists[c]
        inv_lists.append([old[s] if s < len(old) else None for s in perm])

    buckets = []
    s = 0
    while s < e:
        en = ends[s]
        s2 = s
        while s2 < e and ends[s2] == en:
            s2 += 1
        if en < W:
            buckets.append((en, s, s2))
        s = s2

    nc = _build_nc(tuple(starts), tuple(ends), tuple(buckets))

    in_maps = []
    core_rows = []
    for c in range(N_CORES):
        lst = inv_lists[c]

        def gather(a):
            g = np.zeros(e, np.float32)
            for s, t in enumerate(lst):
                if t is not None:
                    g[s] = a[t[0]]
            return g[None, :]

        rows = np.concatenate(
            [np.arange(b * BS, (b + 1) * BS) for b in sorted(core_blocks[c])]
        )
        core_rows.append(rows)
        y_vec = rows.astype(np.float32)[:, None]
        segs = [gather(inv), gather(b_arr), gather(dx), gather(x0),
                gather(coeff)]
        segs += [np.zeros((1, 1), np.float32), col[None, :]]
        row = np.concatenate(segs, axis=1)
        packed = np.broadcast_to(row, (ROWS, row.shape[1])).copy()
        packed[:, 5 * e:5 * e + 1] = y_vec
        in_maps.append({"inp": packed})

    return nc, in_maps, core_rows


def kernel(control_points: np.ndarray, color: np.ndarray) -> np.ndarray:
    nc, in_maps, core_rows = _prepare(control_points, color)
    results = run_bass_kernel_spmd(nc, in_maps, core_ids=list(range(N_CORES))).results
    out = np.empty((H, W, 4), dtype=np.float32)
    for c in range(N_CORES):
        out[core_rows[c]] = results[c]["out"].reshape(ROWS, W, 4)
    return out



# revision 18
# speedup vs baseline: 2.6698x; 2.6698x over previous
"""Bass/Trainium2 kernel for the BayesianVectorRenderer problem.

Renders a closed cubic-Bezier path into a [1024,1024,4] RGBA image via a
soft winding-number accumulation.

Strategy (8 NeuronCores, SPMD, one shared graph):
  - Rows are split into 8 contiguous 128-row bands (one per core).  Since
    every core executes the same instruction stream, per-core time equals
    stream time; the goal is a minimal stream, not per-core balance.
  - Host: sample the Bezier path (512 edges), compute every edge/row
    crossing (xc, W) in fp32 (W folds the reference's soft-t validity and
    edge sign), then express the winding over each 64-px column chunk as
      winding[y, c] = sum_k coef[k, y] * phi_k[c]
    where phi_k[c] = sigmoid(g_k - c) on a 1.25-px anchor grid (plus one
    constant row carrying the far-field step term R).  Each crossing
    contributes to <=9 anchors of 1-2 chunks via precomputed least-squares
    tap weights, linearly interpolated in xc (sup error ~9e-4).
  - Device: per chunk, ONE self-loading fp32 matmul (lhsT=coef [K,128],
    rhs=phi [K,64]) evaluates all sigmoids at once into PSUM.  ScalarE
    then applies alpha = sigmoid(4*winding) straight into the interleaved
    rgba buffer (strided out-AP); DVE broadcast-fills the constant rgb
    channels; output streams out in four 256-column DMA groups overlapped
    with compute.  The stream is DMA-bound (~2MB out per core).
"""

from contextlib import ExitStack

import numpy as np

import concourse.bass as bass
from concourse import mybir
from concourse.bass_utils import run_bass_kernel_spmd

H = 1024
W = 1024
SAMPLES_PER_SEG = 32
N_CORES = 8
ROWS = H // N_CORES      # 128 rows per core
C = 64                   # column chunk width
NCH = W // C             # 16 chunks
M = 12.0                 # sigmoid locality margin (px); sig(-12)=6e-6
DLT = 1.25               # anchor spacing (px)
TAPS = 8                 # anchors per crossing fit
UT = TAPS + 1            # union tap window for xc interpolation
GRID_H = 1.0 / 16.0      # xc fit-interpolation grid step
NGRP = 4                 # output DMA column groups
GW = W // NGRP           # 256 columns per group
CPG = NCH // NGRP        # 4 chunks per group

_BASIS = None


def _sig(z):
    out = np.empty_like(z)
    np.negative(z, out=out)
    np.exp(np.minimum(out, 60.0), out=out)
    out += 1.0
    np.reciprocal(out, out=out)
    return out


def _build_basis():
    """Anchor grid + per-xc-gridpoint least-squares tap weights.

    Returns (K, Phi [K,C] f64, xs, tap0 [NX], alph [NX,TAPS], beta [NX]).
    """
    global _BASIS
    if _BASIS is not None:
        return _BASIS
    pad = (TAPS / 2) * DLT
    g = np.arange(-M - pad, C + M + pad + 1e-9, DLT)
    K = len(g)
    cgrid = np.arange(C, dtype=np.float64)
    Phi = _sig(g[:, None] - cgrid[None, :])
    ones = np.ones(C)
    xs = np.arange(-M, C + M + 1e-9, GRID_H)
    NX = len(xs)
    tap0 = np.zeros(NX, np.int64)
    alph = np.zeros((NX, TAPS), np.float64)
    beta = np.zeros(NX, np.float64)
    for i, xc in enumerate(xs):
        i0 = int(np.floor((xc - g[0]) / DLT)) - (TAPS // 2 - 1)
        i0 = max(0, min(K - TAPS, i0))
        A = np.vstack([Phi[i0:i0 + TAPS], ones])
        target = _sig(xc - cgrid)
        coefs, *_ = np.linalg.lstsq(A.T, target, rcond=None)
        tap0[i] = i0
        alph[i] = coefs[:TAPS]
        beta[i] = coefs[TAPS]
    _BASIS = (K, Phi, xs, tap0, alph, beta)
    return _BASIS


def _sample_bezier(cp: np.ndarray) -> np.ndarray:
    """Faithful fp32 port of reference.sample_bezier_path."""
    cp = cp.astype(np.float32)
    n = cp.shape[0]
    s = (n - 1) // 3
    idx = 3 * np.arange(s)
    p0 = cp[idx][:, None, :]
    p1 = cp[idx + 1][:, None, :]
    p2 = cp[idx + 2][:, None, :]
    p3 = cp[idx + 3][:, None, :]
    t = np.linspace(0.0, 1.0, SAMPLES_PER_SEG, dtype=np.float32)[None, :, None]
    mt = (np.float32(1.0) - t).astype(np.float32)
    pts = (
        (mt * mt * mt) * p0
        + np.float32(3.0) * (mt * mt) * t * p1
        + np.float32(3.0) * mt * (t * t) * p2
        + (t * t * t) * p3
    )
    return pts.reshape(s * SAMPLES_PER_SEG, 2).astype(np.float32)


def _crossings(control_points: np.ndarray):
    """All (row, xc, W) crossings in reference fp32 arithmetic."""
    pts = _sample_bezier(control_points)
    nxt = np.roll(pts, -1, axis=0)
    x0 = pts[:, 0]
    y0 = pts[:, 1]
    dy = (nxt[:, 1] - pts[:, 1]).astype(np.float32)
    dx = (nxt[:, 0] - pts[:, 0]).astype(np.float32)
    coeff = (np.sign(dy) * (np.abs(dy) >= np.float32(1e-6))).astype(np.float32)
    ys = np.arange(H, dtype=np.float32)[:, None]
    t = (ys - y0[None, :]) / (dy[None, :] + np.float32(1e-8))
    valid = _sig(t * np.float32(20.0)) * _sig((np.float32(1.0) - t) * np.float32(20.0))
    Wgt = (coeff[None, :] * valid).astype(np.float32)
    xc = (x0[None, :] + t * dx[None, :]).astype(np.float32)
    keep = (np.abs(Wgt) >= 1e-5) & np.isfinite(xc)
    yy, jj = np.nonzero(keep)
    return yy.astype(np.int64), xc[yy, jj].astype(np.float64), Wgt[yy, jj].astype(np.float64)


def _decompose(yy, xc, Wgt, K, xs, tap0, alph, beta):
    """coef [NCH, K+1, H]: anchor rows 0..K-1, far-field/constant row K."""
    K1 = K + 1
    coef = np.zeros((NCH, K1, H), np.float64)
    # far-field step: +W for every chunk q with q*C + C + M <= xc
    qstep = np.floor((xc - M) / C).astype(np.int64) - 1
    qstep = np.minimum(qstep, NCH - 1)
    sel = qstep >= 0
    stepacc = np.zeros((H, NCH), np.float64)
    np.add.at(stepacc, (yy[sel], qstep[sel]), Wgt[sel])
    R = np.cumsum(stepacc[:, ::-1], axis=1)[:, ::-1]   # [H, NCH]
    # local transition contributions
    qlo = np.maximum(0, (np.floor((xc - C - M) / C) + 1).astype(np.int64))
    qhi = np.minimum(NCH - 1, np.floor((xc + M) / C).astype(np.int64))
    NX = len(xs)
    for q in range(NCH):
        msel = (qlo <= q) & (q <= qhi)
        if not msel.any():
            continue
        xl = xc[msel] - q * C
        yq = yy[msel]
        wq = Wgt[msel]
        pos = (xl + M) / GRID_H
        gi = np.clip(np.floor(pos).astype(np.int64), 0, NX - 2)
        frac = np.clip(pos - gi, 0.0, 1.0)
        t0 = np.minimum(np.minimum(tap0[gi], tap0[gi + 1]), K - UT)
        a = np.zeros((len(xl), UT))
        off0 = tap0[gi] - t0
        off1 = tap0[gi + 1] - t0
        rows = np.arange(len(xl))
        for tp in range(TAPS):
            a[rows, off0 + tp] += alph[gi, tp] * (1.0 - frac)
            a[rows, off1 + tp] += alph[gi + 1, tp] * frac
        a *= wq[:, None]
        for tp in range(UT):
            np.add.at(coef[q], (t0 + tp, yq), a[:, tp])
        np.add.at(R, (yq, q), wq * (beta[gi] * (1 - frac) + beta[gi + 1] * frac))
    coef[:, K, :] = R.T
    return coef


def _build_nc(K1):
    """Build the shared SPMD Bass graph."""
    nc = bass.Bass("TRN2", target_bir_lowering=False, debug=False)
    f32 = mybir.dt.float32
    SIG = mybir.ActivationFunctionType.Sigmoid

    d_coef = nc.declare_dram_parameter("coef", [K1, NCH * ROWS], f32, isOutput=False)
    d_phi = nc.declare_dram_parameter("phi", [K1, C], f32, isOutput=False)
    d_aux = nc.declare_dram_parameter("aux", [ROWS, 4], f32, isOutput=False)
    d_out = nc.declare_dram_parameter("out", [ROWS, W * 4], f32, isOutput=True)

    with ExitStack() as ctx:
        t_coef = ctx.enter_context(nc.sbuf_tensor([K1, NCH * ROWS], f32))
        t_phi = ctx.enter_context(nc.sbuf_tensor([K1, C], f32))
        t_aux = ctx.enter_context(nc.sbuf_tensor([ROWS, 4], f32))
        t_scr = ctx.enter_context(nc.sbuf_tensor([ROWS, 1], f32))
        t_grid = ctx.enter_context(nc.sbuf_tensor([ROWS, W], f32))
        t_alpha = ctx.enter_context(nc.sbuf_tensor([ROWS, W], f32))
        t_rgba = ctx.enter_context(nc.sbuf_tensor([ROWS, W * 4], f32))
        # one PSUM bank per 256-col group: PE must never write a bank
        # ScalarE is reading (PE-W + ScE-R same bank is a hardware fault)
        t_wind = [
            ctx.enter_context(nc.psum_tensor(f"wind{g}", [ROWS, GW], f32))
            for g in range(NGRP)
        ]
        s_aux = ctx.enter_context(nc.semaphore("s_aux"))
        pool_sem = ctx.enter_context(nc.semaphore("pool_sem"))
        v2_sem = ctx.enter_context(nc.semaphore("v2_sem"))
        s_phi = ctx.enter_context(nc.semaphore("s_phi"))
        s_cg = [ctx.enter_context(nc.semaphore(f"s_cg{g}")) for g in range(NGRP)]
        pe_sem = ctx.enter_context(nc.semaphore("pe_sem"))
        act_sem = ctx.enter_context(nc.semaphore("act_sem"))
        dve_sem = ctx.enter_context(nc.semaphore("dve_sem"))
        dma_out = ctx.enter_context(nc.semaphore("dma_out"))
        block = ctx.enter_context(nc.Block())

        rgba4 = t_rgba[:].rearrange("p (c k) -> p c k", k=4)
        wind = [t[:] for t in t_wind]
        coef = t_coef[:]
        phi = t_phi[:]
        aux = t_aux[:]

        @block.sync
        def _(sync):
            sync.dma_start(out=aux, in_=d_aux[:]).then_inc(s_aux, 16)
            sync.dma_start(out=phi, in_=d_phi[:]).then_inc(s_phi, 16)
            for g in range(NGRP):
                cl = g * CPG * ROWS
                ch = (g + 1) * CPG * ROWS
                sync.dma_start(
                    out=coef[:, cl:ch], in_=d_coef[:, cl:ch]
                ).then_inc(s_cg[g], 16)
            for g in range(NGRP):
                sync.wait_ge(v2_sem, g + 1)
                sync.wait_ge(dve_sem, 1)
                sync.dma_start(
                    out=d_out[:, g * GW * 4:(g + 1) * GW * 4],
                    in_=t_rgba[:][:, g * GW * 4:(g + 1) * GW * 4],
                ).then_inc(dma_out, 16)

        @block.tensor
        def _(tensor):
            tensor.wait_ge(s_phi, 16)
            for q in range(NCH):
                g = q // CPG
                if q % CPG == 0:
                    tensor.wait_ge(s_cg[g], 16)
                mm = tensor.matmul(
                    out=wind[g][:, (q % CPG) * C:(q % CPG + 1) * C],
                    lhsT=coef[:, q * ROWS:(q + 1) * ROWS],
                    rhs=phi,
                    start=True,
                    stop=True,
                )
                if q % CPG == CPG - 1:
                    mm.then_inc(pe_sem, 1)

        @block.scalar
        def _(scalar):
            # warm the sigmoid table during the input DMA
            scalar.wait_ge(s_aux, 16)
            scalar.activation(t_scr[:], aux[:, 0:1], SIG)
            for g in range(NGRP):
                scalar.wait_ge(pe_sem, g + 1)
                scalar.activation(
                    t_alpha[:][:, g * GW:(g + 1) * GW],
                    wind[g],
                    SIG,
                    scale=4.0,
                ).then_inc(act_sem, 1)

        @block.gpsimd
        def _(gpsimd):
            gpsimd.iota(
                t_grid[:], pattern=[[1, W]], base=0, channel_multiplier=0,
                allow_small_or_imprecise_dtypes=True,
            ).then_inc(pool_sem, 1)
            gpsimd.wait_ge(s_aux, 16)
            gpsimd.wait_ge(pool_sem, 1)
            A = mybir.AluOpType
            for ch in range(3):
                ins = gpsimd.tensor_scalar(
                    rgba4[:, :, ch], t_grid[:], 0.0, aux[:, ch:ch + 1],
                    op0=A.mult, op1=A.add,
                )
            ins.then_inc(dve_sem, 1)

        @block.vector
        def _(vector):
            for g in range(NGRP):
                vector.wait_ge(act_sem, g + 1)
                vector.tensor_copy(
                    rgba4[:, g * GW:(g + 1) * GW, 3],
                    t_alpha[:][:, g * GW:(g + 1) * GW],
                ).then_inc(v2_sem, 1)

    return nc


def _prepare(control_points: np.ndarray, color: np.ndarray):
    K, Phi, xs, tap0, alph, beta = _build_basis()
    col = np.asarray(color, dtype=np.float32)

    yy, xc, Wgt = _crossings(np.asarray(control_points, dtype=np.float32))
    coef = _decompose(yy, xc, Wgt, K, xs, tap0, alph, beta)  # [NCH, K+1, H]

    # pad the contraction dim to a full 128 partitions (no PE time cost)
    K1 = 128
    coef = np.concatenate(
        [coef, np.zeros((NCH, K1 - (K + 1), H))], axis=1)
    phi_ext = np.concatenate(
        [Phi, np.ones((1, C)), np.zeros((K1 - (K + 1), C))], axis=0
    ).astype(np.float32)
    aux = np.zeros((ROWS, 4), np.float32)
    aux[:, 0:3] = col[None, :]

    nc = _build_nc(K1)

    in_maps = []
    core_rows = []
    for c in range(N_CORES):
        rows = np.arange(c * ROWS, (c + 1) * ROWS)
        core_rows.append(rows)
        lhs = coef[:, :, rows]                      # [NCH, K1, 128]
        lhs = np.ascontiguousarray(
            lhs.transpose(1, 0, 2).reshape(K1, NCH * ROWS)
        ).astype(np.float32)
        in_maps.append({"coef": lhs, "phi": phi_ext, "aux": aux})

    return nc, in_maps, core_rows


def kernel(control_points: np.ndarray, color: np.ndarray) -> np.ndarray:
    nc, in_maps, core_rows = _prepare(control_points, color)
    results = run_bass_kernel_spmd(nc, in_maps, core_ids=list(range(N_CORES))).results
    out = np.empty((H, W, 4), dtype=np.float32)
    for c in range(N_CORES):
        out[core_rows[c]] = results[c]["out"].reshape(ROWS, W, 4)
    return out


# revision 25
# speedup vs baseline: 3.2413x; 1.2141x over previous
"""Bass/Trainium2 kernel for the BayesianVectorRenderer problem.

Renders a closed cubic-Bezier path into a [1024,1024,4] RGBA image via a
soft winding-number accumulation.

Strategy (8 NeuronCores, SPMD, one shared graph):
  - Rows are split into 8 contiguous 128-row bands (one per core).  Since
    every core executes the same instruction stream, per-core time equals
    stream time; the goal is a minimal stream, not per-core balance.
  - Host: sample the Bezier path (512 edges), compute every edge/row
    crossing (xc, W) in fp32 (W folds the reference's soft-t validity and
    edge sign), then express the winding over each 64-px column chunk as
      winding[y, c] = sum_k coef[k, y] * phi_k[c]
    where phi_k[c] = sigmoid(g_k - c) on a 1.25-px anchor grid (plus one
    constant row carrying the far-field step term R).  Each crossing
    contributes to <=9 anchors of 1-2 chunks via precomputed least-squares
    tap weights, linearly interpolated in xc (sup error ~9e-4).
  - Device: per chunk, ONE self-loading fp32 matmul (lhsT=coef [K,128],
    rhs=phi [K,64]) evaluates all sigmoids at once into PSUM.  ScalarE
    then applies alpha = sigmoid(4*winding) straight into the interleaved
    rgba buffer (strided out-AP); DVE broadcast-fills the constant rgb
    channels; output streams out in four 256-column DMA groups overlapped
    with compute.  The stream is DMA-bound (~2MB out per core).
"""

from contextlib import ExitStack

import numpy as np

import concourse.bass as bass
from concourse import mybir
from concourse.bass_utils import run_bass_kernel_spmd

H = 1024
W = 1024
SAMPLES_PER_SEG = 32
N_CORES = 8
ROWS = H // N_CORES      # 128 rows per core
C = 64                   # column chunk width
NCH = W // C             # 16 chunks
M = 12.0                 # sigmoid locality margin (px); sig(-12)=6e-6
DLT = 1.25               # anchor spacing (px)
TAPS = 8                 # anchors per crossing fit
UT = TAPS + 1            # union tap window for xc interpolation
GRID_H = 1.0 / 16.0      # xc fit-interpolation grid step
NGRP = 4                 # output DMA column groups
GW = W // NGRP           # 256 columns per group
CPG = NCH // NGRP        # 4 chunks per group

_BASIS = None


def _sig(z):
    out = np.empty_like(z)
    np.negative(z, out=out)
    np.exp(np.minimum(out, 60.0), out=out)
    out += 1.0
    np.reciprocal(out, out=out)
    return out


def _build_basis():
    """Anchor grid + per-xc-gridpoint least-squares tap weights.

    Returns (K, Phi [K,C] f64, xs, tap0 [NX], alph [NX,TAPS], beta [NX]).
    """
    global _BASIS
    if _BASIS is not None:
        return _BASIS
    pad = (TAPS / 2) * DLT
    g = np.arange(-M - pad, C + M + pad + 1e-9, DLT)
    K = len(g)
    cgrid = np.arange(C, dtype=np.float64)
    Phi = _sig(g[:, None] - cgrid[None, :])
    ones = np.ones(C)
    xs = np.arange(-M, C + M + 1e-9, GRID_H)
    NX = len(xs)
    tap0 = np.zeros(NX, np.int64)
    alph = np.zeros((NX, TAPS), np.float64)
    beta = np.zeros(NX, np.float64)
    lam = 1e-6  # ridge keeps tap weights O(1) so fp16 coef rows are safe
    eye = np.eye(TAPS + 1)
    for i, xc in enumerate(xs):
        i0 = int(np.floor((xc - g[0]) / DLT)) - (TAPS // 2 - 1)
        i0 = max(0, min(K - TAPS, i0))
        A = np.vstack([Phi[i0:i0 + TAPS], ones])
        target = _sig(xc - cgrid)
        coefs = np.linalg.solve(A @ A.T + lam * eye, A @ target)
        tap0[i] = i0
        alph[i] = coefs[:TAPS]
        beta[i] = coefs[TAPS]
    _BASIS = (K, Phi, xs, tap0, alph, beta)
    return _BASIS


def _sample_bezier(cp: np.ndarray) -> np.ndarray:
    """Faithful fp32 port of reference.sample_bezier_path."""
    cp = cp.astype(np.float32)
    n = cp.shape[0]
    s = (n - 1) // 3
    idx = 3 * np.arange(s)
    p0 = cp[idx][:, None, :]
    p1 = cp[idx + 1][:, None, :]
    p2 = cp[idx + 2][:, None, :]
    p3 = cp[idx + 3][:, None, :]
    t = np.linspace(0.0, 1.0, SAMPLES_PER_SEG, dtype=np.float32)[None, :, None]
    mt = (np.float32(1.0) - t).astype(np.float32)
    pts = (
        (mt * mt * mt) * p0
        + np.float32(3.0) * (mt * mt) * t * p1
        + np.float32(3.0) * mt * (t * t) * p2
        + (t * t * t) * p3
    )
    return pts.reshape(s * SAMPLES_PER_SEG, 2).astype(np.float32)


def _crossings(control_points: np.ndarray):
    """All (row, xc, W) crossings in reference fp32 arithmetic."""
    pts = _sample_bezier(control_points)
    nxt = np.roll(pts, -1, axis=0)
    x0 = pts[:, 0]
    y0 = pts[:, 1]
    dy = (nxt[:, 1] - pts[:, 1]).astype(np.float32)
    dx = (nxt[:, 0] - pts[:, 0]).astype(np.float32)
    coeff = (np.sign(dy) * (np.abs(dy) >= np.float32(1e-6))).astype(np.float32)
    ys = np.arange(H, dtype=np.float32)[:, None]
    t = (ys - y0[None, :]) / (dy[None, :] + np.float32(1e-8))
    valid = _sig(t * np.float32(20.0)) * _sig((np.float32(1.0) - t) * np.float32(20.0))
    Wgt = (coeff[None, :] * valid).astype(np.float32)
    xc = (x0[None, :] + t * dx[None, :]).astype(np.float32)
    keep = (np.abs(Wgt) >= 1e-5) & np.isfinite(xc)
    yy, jj = np.nonzero(keep)
    return yy.astype(np.int64), xc[yy, jj].astype(np.float64), Wgt[yy, jj].astype(np.float64)


def _decompose(yy, xc, Wgt, K, xs, tap0, alph, beta):
    """coef [NCH, K+1, H]: anchor rows 0..K-1, far-field/constant row K."""
    K1 = K + 1
    coef = np.zeros((NCH, K1, H), np.float64)
    # far-field step: +W for every chunk q with q*C + C + M <= xc
    qstep = np.floor((xc - M) / C).astype(np.int64) - 1
    qstep = np.minimum(qstep, NCH - 1)
    sel = qstep >= 0
    stepacc = np.zeros((H, NCH), np.float64)
    np.add.at(stepacc, (yy[sel], qstep[sel]), Wgt[sel])
    R = np.cumsum(stepacc[:, ::-1], axis=1)[:, ::-1]   # [H, NCH]
    # local transition contributions
    qlo = np.maximum(0, (np.floor((xc - C - M) / C) + 1).astype(np.int64))
    qhi = np.minimum(NCH - 1, np.floor((xc + M) / C).astype(np.int64))
    NX = len(xs)
    for q in range(NCH):
        msel = (qlo <= q) & (q <= qhi)
        if not msel.any():
            continue
        xl = xc[msel] - q * C
        yq = yy[msel]
        wq = Wgt[msel]
        pos = (xl + M) / GRID_H
        gi = np.clip(np.floor(pos).astype(np.int64), 0, NX - 2)
        frac = np.clip(pos - gi, 0.0, 1.0)
        t0 = np.minimum(np.minimum(tap0[gi], tap0[gi + 1]), K - UT)
        a = np.zeros((len(xl), UT))
        off0 = tap0[gi] - t0
        off1 = tap0[gi + 1] - t0
        rows = np.arange(len(xl))
        for tp in range(TAPS):
            a[rows, off0 + tp] += alph[gi, tp] * (1.0 - frac)
            a[rows, off1 + tp] += alph[gi + 1, tp] * frac
        a *= wq[:, None]
        for tp in range(UT):
            np.add.at(coef[q], (t0 + tp, yq), a[:, tp])
        np.add.at(R, (yq, q), wq * (beta[gi] * (1 - frac) + beta[gi + 1] * frac))
    coef[:, K, :] = R.T
    return coef


def _build_nc(K1):
    """Build the shared SPMD Bass graph."""
    nc = bass.Bass("TRN2", target_bir_lowering=False, debug=False)
    f32 = mybir.dt.float32
    f16 = mybir.dt.float16
    SIG = mybir.ActivationFunctionType.Sigmoid

    d_coef = nc.declare_dram_parameter("coef", [K1, NCH * ROWS], f16, isOutput=False)
    d_phi = nc.declare_dram_parameter("phi", [K1, C], f16, isOutput=False)
    d_aux = nc.declare_dram_parameter("aux", [ROWS, 4], f32, isOutput=False)
    d_out = nc.declare_dram_parameter("out", [ROWS, W * 4], f32, isOutput=True)

    with ExitStack() as ctx:
        t_coef = ctx.enter_context(nc.sbuf_tensor([K1, NCH * ROWS], f16))
        t_phi = ctx.enter_context(nc.sbuf_tensor([K1, C], f16))
        t_aux = ctx.enter_context(nc.sbuf_tensor([ROWS, 4], f32))
        t_scr = ctx.enter_context(nc.sbuf_tensor([ROWS, 1], f32))
        t_alpha = ctx.enter_context(nc.sbuf_tensor([ROWS, W], f32))
        t_rgba = ctx.enter_context(nc.sbuf_tensor([ROWS, W * 4], f32))
        # one PSUM bank per 256-col group: PE must never write a bank
        # ScalarE is reading (PE-W + ScE-R same bank is a hardware fault)
        t_wind = [
            ctx.enter_context(nc.psum_tensor(f"wind{g}", [ROWS, GW], f32))
            for g in range(NGRP)
        ]
        s_aux = ctx.enter_context(nc.semaphore("s_aux"))
        v2_sem = ctx.enter_context(nc.semaphore("v2_sem"))
        s_phi = ctx.enter_context(nc.semaphore("s_phi"))
        s_cg = [ctx.enter_context(nc.semaphore(f"s_cg{g}")) for g in range(NGRP)]
        pe_sem = ctx.enter_context(nc.semaphore("pe_sem"))
        act_sem = ctx.enter_context(nc.semaphore("act_sem"))
        dma_out = ctx.enter_context(nc.semaphore("dma_out"))
        block = ctx.enter_context(nc.Block())

        rgba4 = t_rgba[:].rearrange("p (c k) -> p c k", k=4)
        wind = [t[:] for t in t_wind]
        coef = t_coef[:]
        phi = t_phi[:]
        aux = t_aux[:]

        @block.sync
        def _(sync):
            sync.dma_start(out=aux, in_=d_aux[:]).then_inc(s_aux, 16)
            sync.dma_start(out=phi, in_=d_phi[:]).then_inc(s_phi, 16)
            for g in range(NGRP):
                cl = g * CPG * ROWS
                ch = (g + 1) * CPG * ROWS
                sync.dma_start(
                    out=coef[:, cl:ch], in_=d_coef[:, cl:ch]
                ).then_inc(s_cg[g], 16)
            for g in range(NGRP):
                sync.wait_ge(v2_sem, g + 1)
                sync.dma_start(
                    out=d_out[:, g * GW * 4:(g + 1) * GW * 4],
                    in_=t_rgba[:][:, g * GW * 4:(g + 1) * GW * 4],
                ).then_inc(dma_out, 16)

        @block.tensor
        def _(tensor):
            tensor.wait_ge(s_phi, 16)
            for q in range(NCH):
                g = q // CPG
                if q % CPG == 0:
                    tensor.wait_ge(s_cg[g], 16)
                mm = tensor.matmul(
                    out=wind[g][:, (q % CPG) * C:(q % CPG + 1) * C],
                    lhsT=coef[:, q * ROWS:(q + 1) * ROWS],
                    rhs=phi,
                    start=True,
                    stop=True,
                )
                if q % CPG == CPG - 1:
                    mm.then_inc(pe_sem, 1)

        @block.scalar
        def _(scalar):
            # warm the sigmoid table during the input DMA
            scalar.wait_ge(s_aux, 16)
            scalar.activation(t_scr[:], aux[:, 0:1], SIG)
            for g in range(NGRP):
                scalar.wait_ge(pe_sem, g + 1)
                scalar.activation(
                    t_alpha[:][:, g * GW:(g + 1) * GW],
                    wind[g],
                    SIG,
                    scale=4.0,
                ).then_inc(act_sem, 1)

        @block.vector
        def _(vector):
            vector.wait_ge(s_aux, 16)
            for g in range(NGRP):
                for ch in range(3):
                    ins = vector.tensor_copy(
                        rgba4[:, g * GW:(g + 1) * GW, ch],
                        aux[:, ch:ch + 1].broadcast_to((ROWS, GW)),
                    )
                vector.wait_ge(act_sem, g + 1)
                vector.tensor_copy(
                    rgba4[:, g * GW:(g + 1) * GW, 3],
                    t_alpha[:][:, g * GW:(g + 1) * GW],
                ).then_inc(v2_sem, 1)

    return nc


def _prepare(control_points: np.ndarray, color: np.ndarray):
    K, Phi, xs, tap0, alph, beta = _build_basis()
    col = np.asarray(color, dtype=np.float32)

    yy, xc, Wgt = _crossings(np.asarray(control_points, dtype=np.float32))
    coef = _decompose(yy, xc, Wgt, K, xs, tap0, alph, beta)  # [NCH, K+1, H]

    # fp16 operand pack: anchor rows direct, far-field R row split hi/lo
    # (|R| up to ~40 would lose too much in a single fp16 row)
    K1 = K + 2
    Rrow = coef[:, K, :]
    Rhi = Rrow.astype(np.float16).astype(np.float64)
    packed = np.concatenate(
        [coef[:, :K, :], Rhi[:, None, :], (Rrow - Rhi)[:, None, :]], axis=1)
    coef = packed
    phi_ext = np.concatenate(
        [Phi, np.ones((1, C)), np.ones((1, C))], axis=0
    ).astype(np.float16)
    aux = np.zeros((ROWS, 4), np.float32)
    aux[:, 0:3] = col[None, :]

    nc = _build_nc(K1)

    in_maps = []
    core_rows = []
    for c in range(N_CORES):
        rows = np.arange(c * ROWS, (c + 1) * ROWS)
        core_rows.append(rows)
        lhs = coef[:, :, rows]                      # [NCH, K1, 128]
        lhs = np.ascontiguousarray(
            lhs.transpose(1, 0, 2).reshape(K1, NCH * ROWS)
        ).astype(np.float16)
        in_maps.append({"coef": lhs, "phi": phi_ext, "aux": aux})

    return nc, in_maps, core_rows


def kernel(control_points: np.ndarray, color: np.ndarray) -> np.ndarray:
    nc, in_maps, core_rows = _prepare(control_points, color)
    results = run_bass_kernel_spmd(nc, in_maps, core_ids=list(range(N_CORES))).results
    out = np.empty((H, W, 4), dtype=np.float32)
    for c in range(N_CORES):
        out[core_rows[c]] = results[c]["out"].reshape(ROWS, W, 4)
    return out
